# revision 54
# baseline (speedup 1.0000x reference)
"""Trainium2 Bass kernel for a 2-layer GCN + TopK pooling + mean pool + linear head.

Reference computation (see problem):
  x = relu(gcn_conv(x, edge_index, W0, b0))
  x = relu(gcn_conv(x, edge_index, W1, b1))
  score = tanh((x @ pool_w) / ||pool_w||); top-K=250 of 500 per graph
  pooled = mean over kept nodes of (x * score); logits = pooled @ W_lin + b_lin
  out = log_softmax(logits)

Sharding: data-parallel over graphs. 104 padded graphs, 13 per core.
Each core aggregates (gather + one-hot matmul scatter) only the edges whose
*target* node lives in its 6500-node slab. Self-loops are appended to the
edge list so the GCN "+I" term needs no special case.

v2 design notes (vs the first working version):
  - Degrees and D^-1/2 factors are host-precomputed. Layer 1 gathers rows of
    xs = dis*x directly (W0 is applied AFTER aggregation: diag(dis)(A+I)
    diag(dis) (x W0) = [diag(dis)(A+I)(diag(dis) x)] W0), killing the
    on-device degree pass, the degree AllGather and the g1 table round-trip.
  - One-hot scatter matrices are built one DVE instruction per *block*
    (C chunks wide) instead of per chunk, via broadcast access patterns.
  - Edges are sorted by source within each (block, chunk) bucket so the
    indirect-gather descriptors hit adjacent DRAM rows when possible.
Cross-core exchange: one AllGather of the layer-2 gather table ([52000,64]).
"""

import os
import sys

for _p in ("/opt/trn_rl_repo", "/root/.axon_site/_ro/trn_rl_repo"):
    if os.path.isdir(_p) and _p not in sys.path:
        sys.path.insert(0, _p)

import dataclasses

import numpy as np

import concourse.bacc as bacc
import concourse.bass as bass
import concourse.mybir as mybir
import concourse.tile as tile
from concourse.bass_utils import run_bass_kernel_spmd  # noqa: F401  (public API)
from concourse.masks import make_identity
from concourse.tile import add_dep_helper


def _dep(after, befores):
    for b in befores:
        add_dep_helper(after.ins, b.ins, sync=True, reason="dram raw order")

# ---- problem constants (hardcoded per contract) ----
N = 50000          # real nodes
E = 800000         # edges
G = 100            # graphs
NPG = 500          # nodes per graph
K = 250            # top-k per graph
D = 64
OUT = 10
NCORES = 8
BLK = int(os.environ.get("GNN_BLK", "50"))  # nodes per aggregation block
GPC = 13           # graphs per core (padded to 104 graphs)
NPAD = NCORES * GPC * NPG      # 52000
NLOC = NPAD // NCORES          # 6500
LBLK = NLOC // BLK             # local blocks per core (130 @ BLK=50)
NBLK = NCORES * LBLK           # global blocks
CH = 128                       # edges per chunk (matmul contraction size)
QB = NPG // BLK                # blocks per group = one graph per group
BPG = NPG // BLK               # blocks per graph
WSUB = QB if QB * D * 4 <= 2048 else QB // 2   # W-matmul psum sub-group

F32 = mybir.dt.float32
I32 = mybir.dt.int32

# gather-table / one-hot dtype: float32 (exact) or bfloat16 (2x DMA, 4x PE)
DT_TBL = mybir.dt.bfloat16 if os.environ.get("GNN_TBL_BF16", "1") == "1" else F32
# weight dtype for the dense matmuls
DT_W = mybir.dt.bfloat16 if os.environ.get("GNN_W_BF16", "1") == "1" else F32
# one-hot matrix dtype (0/1 are exact in every float dtype; fp8 halves the
# SBUF footprint so the full set fits and layer 2 can reuse layer 1's)
DT_OH = (mybir.dt.float8e4 if os.environ.get("GNN_OH_FP8", "1") == "1"
         else DT_TBL)


def _preprocess(edge_index):
    """Bucket edges (plus self-loops) by target block; build per-core
    [128, TC] index/column arrays laid out chunk-major. Also compute the
    symmetric-normalization factors dis = (deg)^-1/2 on the host."""
    row = np.asarray(edge_index[0], dtype=np.int64)
    col = np.asarray(edge_index[1], dtype=np.int64)
    loops = np.arange(NPAD, dtype=np.int64)
    rows_all = np.concatenate([row, loops])
    cols_all = np.concatenate([col, loops])

    deg = np.bincount(cols_all, minlength=NPAD).astype(np.float64)
    dis = (1.0 / np.sqrt(deg)).astype(np.float32)   # deg >= 1 (self-loops)

    blk = (cols_all // BLK).astype(np.int64)          # global target block
    col_loc = (cols_all % BLK).astype(np.int64)

    # sort by (block, source) - source-sorted order improves DMA locality
    order = np.lexsort((rows_all, blk))
    row_s = rows_all[order]
    colloc_s = col_loc[order]

    counts = np.bincount(blk, minlength=NBLK)
    cnts = counts.reshape(NCORES, LBLK)
    C_j = np.maximum(1, -(-cnts.max(axis=0) // CH))   # chunks per local block
    TC = int(C_j.sum())
    starts = np.zeros(LBLK, np.int64)
    starts[1:] = np.cumsum(C_j)[:-1]

    idx_row = np.zeros((NCORES, 128, 2 * TC), np.int32)
    col_lcl = np.full((NCORES, 128, TC), float(BLK), np.float32)  # pad -> no match
    bounds = np.concatenate([[0], np.cumsum(counts)])
    blk_sorted = blk[order]
    rank = np.arange(len(blk_sorted)) - bounds[blk_sorted]  # rank within block
    kk = blk_sorted // LBLK
    jj = blk_sorted % LBLK
    pp = rank % CH
    cc = starts[jj] + rank // CH
    idx_row[kk, pp, cc] = row_s            # layer-1 gather: rows of xs
    # layer-2 gather: rows of g2t, whose halves are AllGathered separately
    # (g2t[:NPAD//2] = concat_c g2l_c[:H], g2t[NPAD//2:] = concat_c g2l_c[H:])
    H = NLOC // 2
    c_src = row_s // NLOC
    n_loc = row_s % NLOC
    row2 = np.where(n_loc < H, c_src * H + n_loc,
                    NPAD // 2 + c_src * H + (n_loc - H))
    idx_row[kk, pp, TC + cc] = row2
    col_lcl[kk, pp, cc] = colloc_s
    return idx_row, col_lcl, tuple(int(c) for c in C_j), TC, dis


def _topk_mask(tc, out, in_, k_to_choose, min_val):
    """Mask of 1s where the top-k values per partition are (from
    concourse.kernels.top_k, inlined to fix a decorator/signature clash)."""
    nc = tc.nc
    KA = 8
    with tc.tile_pool(name="topk_sbuf", bufs=2) as sbuf_pool:
        tensor_on = in_
        for k_on in range(0, k_to_choose, KA):
            k_max = min(k_on + KA, k_to_choose)
            k_this = k_max - k_on
            mx = sbuf_pool.tile([in_.shape[0], KA], in_.dtype, tag="topk_mx")
            nc.vector.max(out=mx[:], in_=tensor_on)
            if k_this < KA:
                nc.vector.memset(mx[:, k_this:], min_val)
            nc.vector.match_replace(out=out, in_to_replace=mx[:],
                                    in_values=tensor_on, imm_value=min_val)
            tensor_on = out
        nc.vector.tensor_sub(out=out, in0=in_, in1=out)
        nc.vector.tensor_scalar_min(out, out, 1.0)


def _build_program(C_j, TC, sim=False, reps=1, zero_bias=False):
    # sim=True: single-core timing-model build - collectives replaced by
    # local DMA copies (TimelineSim can't model collectives).
    nc = bacc.Bacc("TRN2", target_bir_lowering=False, debug=False,
                   num_devices=1 if sim else NCORES)

    xs = nc.dram_tensor("xs", [NPAD, D], DT_TBL, kind="ExternalInput").ap()
    W0 = nc.dram_tensor("W0", [D, D], DT_W, kind="ExternalInput").ap()
    W1 = nc.dram_tensor("W1", [D, D], DT_W, kind="ExternalInput").ap()
    Wl = nc.dram_tensor("Wl", [D, OUT], F32, kind="ExternalInput").ap()
    b0b = nc.dram_tensor("b0b", [128, D], F32, kind="ExternalInput").ap()
    b1b = nc.dram_tensor("b1b", [128, D], F32, kind="ExternalInput").ap()
    pwb = nc.dram_tensor("pwb", [128, D], F32, kind="ExternalInput").ap()
    blb = nc.dram_tensor("blb", [128, OUT], F32, kind="ExternalInput").ap()
    dislT = nc.dram_tensor("dislT", [BLK, LBLK], F32, kind="ExternalInput").ap()
    idxs = nc.dram_tensor("idxs", [128, 2 * TC], I32, kind="ExternalInput").ap()
    cols = nc.dram_tensor("cols", [128, TC], DT_TBL, kind="ExternalInput").ap()
    outp = nc.dram_tensor("out", [GPC, OUT], F32, kind="ExternalOutput").ap()

    g2l = nc.dram_tensor("g2l", [NLOC, D], DT_TBL,
                         kind="ExternalOutput" if os.environ.get("GNN_DBG_G2")
                         else "Internal").ap()
    g2t = nc.dram_tensor("g2t", [NPAD, D], DT_TBL, kind="Internal",
                         addr_space="Shared").ap()
    scd = nc.dram_tensor("scd", [NLOC], DT_TBL, kind="Internal").ap()
    wd = nc.dram_tensor("wd", [NLOC], DT_TBL, kind="Internal").ap()

    starts = [0] * LBLK
    for j in range(1, LBLK):
        starts[j] = starts[j - 1] + C_j[j - 1]
    Cmax = max(C_j)

    rg = [list(range(NCORES))]

    def bcast_mid(ap2d, nmid):
        """[P, W] tile -> [P, nmid, W] AP with step-0 middle dim."""
        a = ap2d.ap
        return dataclasses.replace(ap2d, ap=[list(a[0]), [0, nmid],
                                             list(a[1])])

    with tile.TileContext(nc) as tc:
        with (
            tc.tile_pool(name="const", bufs=1) as cpool,
            tc.tile_pool(name="slab", bufs=1) as slab,
            tc.tile_pool(name="gat", bufs=6) as gatpool,
            tc.tile_pool(name="tmp", bufs=4) as tpool,
            tc.tile_pool(name="ps_agg", bufs=3, space="PSUM") as ps_agg,
            tc.tile_pool(name="ps_mm", bufs=2, space="PSUM") as ps_mm,
            tc.tile_pool(name="ps_tr", bufs=2, space="PSUM") as ps_tr,
            tc.tile_pool(name="ps_acc", bufs=1, space="PSUM") as ps_acc,
        ):
            # ---- constants ----
            W0sb = cpool.tile([D, D], DT_W)
            W1sb = cpool.tile([D, D], DT_W)
            Wlsb = cpool.tile([D, OUT], F32)
            b0sb = cpool.tile([128, D], F32)
            b1sb = cpool.tile([128, D], F32)
            pwsb = cpool.tile([128, D], F32)
            blsb = cpool.tile([128, OUT], F32)
            dissb = cpool.tile([BLK, LBLK], F32)
            nc.sync.dma_start(out=W0sb[:], in_=W0[:])
            nc.sync.dma_start(out=W1sb[:], in_=W1[:])
            nc.sync.dma_start(out=Wlsb[:], in_=Wl[:])
            nc.sync.dma_start(out=b0sb[:], in_=b0b[:])
            nc.sync.dma_start(out=b1sb[:], in_=b1b[:])
            nc.sync.dma_start(out=pwsb[:], in_=pwb[:])
            nc.sync.dma_start(out=blsb[:], in_=blb[:])
            nc.sync.dma_start(out=dissb[:], in_=dislT[:])

            for _rep in range(reps):
                idx1_sb = slab.tile([128, TC], I32)
                idx2_sb = slab.tile([128, TC], I32)
                col_sb = slab.tile([128, TC], DT_TBL)
                nc.sync.dma_start(out=idx1_sb[:], in_=idxs[:, :TC])
                nc.sync.dma_start(out=idx2_sb[:], in_=idxs[:, TC:])
                nc.sync.dma_start(out=col_sb[:], in_=cols[:])

                iota_i = cpool.tile([128, BLK], I32)
                iota_f = cpool.tile([128, BLK], DT_TBL)
                nc.gpsimd.iota(iota_i[:], pattern=[[1, BLK]], base=0,
                               channel_multiplier=0)
                nc.vector.tensor_copy(iota_f[:], iota_i[:])
                ones_f = cpool.tile([128, 1], F32)
                nc.vector.memset(ones_f[:], 1.0)
                ident = cpool.tile([128, 128], F32)
                make_identity(nc, ident[:])

                aggT = slab.tile([D, NLOC], DT_TBL)       # transposed agg
                g2slab = slab.tile([BLK, LBLK * D], DT_TBL)
                out2_slab = slab.tile([BLK, LBLK * D], F32)
                sc_slab = slab.tile([BLK, LBLK], F32)
                dissq = cpool.tile([BLK, LBLK], F32)      # dis^2 per node
                nc.vector.tensor_mul(dissq[:], dissb[:], dissb[:])

                oh_full = slab.tile([128, TC * BLK], DT_OH)

                def layer_pass(table, idx_sb, table_deps, Wsb, post_cb,
                               build_oh):
                    """Per group of QB blocks: gather rows of `table`,
                    scatter-sum them via one-hot matmuls (gathered chunk is
                    the stationary operand -> cheap LDWEIGHTS; the [64, BLK]
                    product is the aggregation pre-transposed, exactly the
                    lhsT the W matmul wants), then apply W and hand the psum
                    to post_cb(jq, psh). disl scaling happens in post_cb."""
                    for jq in range(0, LBLK, QB):
                        o0 = starts[jq]
                        ctot = sum(C_j[jq:jq + QB])
                        gat = gatpool.tile([128, QB * Cmax * D], DT_TBL,
                                           tag="gat")
                        g_ins = nc.gpsimd.indirect_dma_start(
                            out=gat[:, :ctot * D],
                            out_offset=None,
                            in_=table[:],
                            in_offset=bass.IndirectOffsetOnAxis(
                                ap=idx_sb[:, o0:o0 + ctot], axis=0),
                        )
                        _dep(g_ins, table_deps)
                        # one-hot scatter matrices (shared by both layers:
                        # same edges), one DVE instruction per block
                        if build_oh:
                            for bi in range(QB):
                                j = jq + bi
                                cj = C_j[j]
                                nc.vector.tensor_tensor(
                                    out=oh_full[:, starts[j] * BLK:
                                                (starts[j] + cj) * BLK]
                                        .rearrange("p (c b) -> p c b", b=BLK),
                                    in0=col_sb[:, starts[j]:starts[j] + cj]
                                        .to_broadcast([128, cj, BLK]),
                                    in1=bcast_mid(iota_f[:], cj),
                                    op=mybir.AluOpType.is_equal)
                        ps4 = ps_agg.tile([D, QB * BLK], F32, tag="agg")
                        for bi in range(QB):
                            j = jq + bi
                            coff = starts[j] - o0
                            for c in range(C_j[j]):
                                gc = starts[j] + c
                                nc.tensor.matmul(
                                    ps4[:, bi * BLK:(bi + 1) * BLK],
                                    lhsT=gat[:, (coff + c) * D:
                                             (coff + c + 1) * D],
                                    rhs=oh_full[:, gc * BLK:
                                                (gc + 1) * BLK],
                                    start=(c == 0), stop=(c == C_j[j] - 1))
                        # psum -> sbuf copy on the (mostly idle) scalar
                        # engine so the DVE FIFO never head-blocks on it
                        nc.scalar.activation(
                            aggT[:, jq * BLK:(jq + QB) * BLK], ps4[:],
                            mybir.ActivationFunctionType.Copy)
                        pshs = []
                        for s0 in range(0, QB, WSUB):
                            psh = ps_mm.tile([BLK, WSUB * D], F32, tag="mm")
                            for bi in range(WSUB):
                                j = jq + s0 + bi
                                nc.tensor.matmul(
                                    psh[:, bi * D:(bi + 1) * D],
                                    lhsT=aggT[:, j * BLK:(j + 1) * BLK],
                                    rhs=Wsb[:], start=True, stop=True)
                            pshs.append((jq + s0, psh))
                        post_cb(jq, pshs)

                def scale_dis(dst_ap, src_ap, sq):
                    nc.vector.tensor_tensor(
                        out=dst_ap.rearrange("p (b d) -> p b d", d=D),
                        in0=src_ap.rearrange("p (b d) -> p b d", d=D),
                        in1=dissb[:, sq:sq + WSUB].to_broadcast(
                            [BLK, WSUB, D]),
                        op=mybir.AluOpType.mult)

                def add_bias(dst_ap, src_ap, bsb):
                    nc.vector.tensor_tensor(
                        out=dst_ap.rearrange("p (b d) -> p b d", d=D),
                        in0=src_ap.rearrange("p (b d) -> p b d", d=D),
                        in1=bcast_mid(bsb[:BLK, :], WSUB),
                        op=mybir.AluOpType.add)

                # ---- layer 1: agg = (A+I) xs (xs = dis*x pre-scaled);
                #      out1 = relu(dis*agg W0 + b0); g2 = dis * out1.
                #      The g2 AllGather is split in halves so the first
                #      half's exchange overlaps the second half's compute.
                HALF = LBLK // 2
                g2_stores = []
                ag_list = []

                def l1_post(jq, pshs):
                    for sq, psh in pshs:
                        hb_sb = tpool.tile([BLK, WSUB * D], F32, tag="hb")
                        if zero_bias:
                            # g2 = dis * relu(dis * (agg W0)) =
                            #      dis^2 * relu(agg W0)   (dis > 0)
                            nc.scalar.activation(
                                hb_sb[:], psh[:],
                                mybir.ActivationFunctionType.Relu)
                            nc.vector.tensor_tensor(
                                out=g2slab[:, sq * D:(sq + WSUB) * D]
                                    .rearrange("p (b d) -> p b d", d=D),
                                in0=hb_sb[:].rearrange(
                                    "p (b d) -> p b d", d=D),
                                in1=dissq[:, sq:sq + WSUB].to_broadcast(
                                    [BLK, WSUB, D]),
                                op=mybir.AluOpType.mult)
                            continue
                        scale_dis(hb_sb[:], psh[:], sq)
                        add_bias(hb_sb[:], hb_sb[:], b0sb)
                        nc.scalar.activation(
                            hb_sb[:], hb_sb[:],
                            mybir.ActivationFunctionType.Relu)
                        nc.vector.tensor_tensor(
                            out=g2slab[:, sq * D:(sq + WSUB) * D].rearrange(
                                "p (b d) -> p b d", d=D),
                            in0=hb_sb[:].rearrange("p (b d) -> p b d", d=D),
                            in1=dissb[:, sq:sq + WSUB].to_broadcast(
                                [BLK, WSUB, D]),
                            op=mybir.AluOpType.mult)
                    g2_stores.append(nc.sync.dma_start(
                        out=g2l.rearrange("(b p) d -> p b d", p=BLK)
                            [:, jq:jq + QB, :],
                        in_=g2slab[:, jq * D:(jq + QB) * D].rearrange(
                            "p (b d) -> p b d", d=D)))
                    sq = jq
                    if sq < HALF <= sq + QB:  # first-half blocks all stored
                        if sim:
                            ag = nc.gpsimd.dma_start(
                                out=g2t[:HALF * BLK, :],
                                in_=g2l[:HALF * BLK, :])
                        else:
                            ag = nc.gpsimd.collective_compute(
                                "AllGather", mybir.AluOpType.bypass,
                                replica_groups=rg,
                                ins=[g2l[:HALF * BLK, :]],
                                outs=[g2t[:NPAD // 2, :]])
                        _dep(ag, g2_stores)
                        ag_list.append(ag)

                layer_pass(xs, idx1_sb, [], W0sb, l1_post, build_oh=True)
                if sim:
                    ag = nc.gpsimd.dma_start(out=g2t[HALF * BLK:NLOC, :],
                                             in_=g2l[HALF * BLK:, :])
                else:
                    ag = nc.gpsimd.collective_compute(
                        "AllGather", mybir.AluOpType.bypass,
                        replica_groups=rg, ins=[g2l[HALF * BLK:, :]],
                        outs=[g2t[NPAD // 2:, :]])
                _dep(ag, g2_stores)
                ag_list.append(ag)

                # ---- layer 2: agg2 = (A+I) g2 ; out2 = relu(dis*agg2 W1+b1)
                #      (scores are computed per group so they overlap with
                #      the remaining groups' aggregation work)
                sc_view = scd.rearrange("(j p) -> j p", p=BLK)
                w_stores = []

                def l2_post(jq, pshs):
                    for sq, psh in pshs:
                        if zero_bias:
                            hb_sb = tpool.tile([BLK, WSUB * D], F32,
                                               tag="hb")
                            nc.scalar.activation(
                                hb_sb[:], psh[:],
                                mybir.ActivationFunctionType.Relu)
                            scale_dis(
                                out2_slab[:, sq * D:(sq + WSUB) * D],
                                hb_sb[:], sq)
                        else:
                            scale_dis(out2_slab[:, sq * D:(sq + WSUB) * D],
                                      psh[:], sq)
                            add_bias(out2_slab[:, sq * D:(sq + WSUB) * D],
                                     out2_slab[:, sq * D:(sq + WSUB) * D],
                                     b1sb)
                            nc.scalar.activation(
                                out2_slab[:, sq * D:(sq + WSUB) * D],
                                out2_slab[:, sq * D:(sq + WSUB) * D],
                                mybir.ActivationFunctionType.Relu)
                    # per-group scores: sc[p, j] = sum_d out2[p, j, d]*pw[d]
                    grp = out2_slab[:, jq * D:(jq + QB) * D]
                    sct = tpool.tile([BLK, QB * D], F32, tag="sct")
                    nc.vector.tensor_tensor(
                        out=sct[:].rearrange("p (b d) -> p b d", d=D),
                        in0=grp.rearrange("p (b d) -> p b d", d=D),
                        in1=bcast_mid(pwsb[:BLK, :], QB),
                        op=mybir.AluOpType.mult)
                    nc.vector.tensor_reduce(
                        sc_slab[:, jq:jq + QB],
                        sct[:].rearrange("p (b d) -> p b d", d=D),
                        axis=mybir.AxisListType.X, op=mybir.AluOpType.add)
                layer_pass(g2t, idx2_sb, ag_list, W1sb, l2_post,
                           build_oh=False)
                # single end-of-layer topk: injecting it mid-L2 delays the
                # DVE one-hot builds that pace the pipeline (measured slower)
                HJ = LBLK // 2
                nc.scalar.activation(sc_slab[:], sc_slab[:],
                                     mybir.ActivationFunctionType.Tanh)
                pst = ps_tr.tile([HJ, 2 * BLK], F32, tag="tr")
                nc.tensor.transpose(pst[:, :BLK], sc_slab[:, :HJ],
                                    ident[:BLK, :BLK])
                nc.tensor.transpose(pst[:, BLK:], sc_slab[:, HJ:],
                                    ident[:BLK, :BLK])
                scT = tpool.tile([HJ, 2 * BLK], DT_TBL, tag="scT")
                nc.vector.tensor_copy(scT[:], pst[:])
                st_sc = [
                    nc.sync.dma_start(out=sc_view[:HJ], in_=scT[:, :BLK]),
                    nc.sync.dma_start(out=sc_view[HJ:], in_=scT[:, BLK:]),
                ]
                sc13 = slab.tile([GPC, NPG], DT_TBL)
                ld_sc = nc.sync.dma_start(
                    out=sc13[:], in_=scd.rearrange("(g n) -> g n", n=NPG))
                _dep(ld_sc, st_sc)
                mask13 = slab.tile([GPC, NPG], DT_TBL)
                _topk_mask(tc, mask13[:], sc13[:], K, min_val=-4.0)
                w13 = slab.tile([GPC, NPG], DT_TBL)
                nc.vector.tensor_mul(w13[:], mask13[:], sc13[:])
                w_stores.append(nc.sync.dma_start(
                    out=wd.rearrange("(g n) -> g n", n=NPG), in_=w13[:]))
                wT = tpool.tile([HJ, 2 * BLK], DT_TBL, tag="wT")
                w_view = wd.rearrange("(j p) -> j p", p=BLK)
                ld_w = [
                    nc.sync.dma_start(out=wT[:, :BLK], in_=w_view[:HJ]),
                    nc.sync.dma_start(out=wT[:, BLK:], in_=w_view[HJ:]),
                ]
                _dep(ld_w[0], w_stores)
                _dep(ld_w[1], w_stores)
                wTf = tpool.tile([HJ, 2 * BLK], F32, tag="wTf")
                nc.vector.tensor_copy(wTf[:], wT[:])
                psw = ps_tr.tile([BLK, LBLK], F32, tag="tr")
                nc.tensor.transpose(psw[:, :HJ], wTf[:, :BLK],
                                    ident[:HJ, :HJ])
                nc.tensor.transpose(psw[:, HJ:], wTf[:, BLK:],
                                    ident[:HJ, :HJ])
                w_slab = slab.tile([BLK, LBLK], F32)
                nc.vector.tensor_copy(w_slab[:], psw[:])

                # pooledT[d, gr] = sum_n w[n] * out2[n, d]: the w multiply
                # rides in the matmul as the [BLK, 1] moving operand
                psp = ps_acc.tile([D, GPC], F32, tag="acc")
                for j in range(LBLK):
                    gr = j // BPG
                    nc.tensor.matmul(psp[:, gr:gr + 1],
                                     lhsT=out2_slab[:, j * D:(j + 1) * D],
                                     rhs=w_slab[:, j:j + 1],
                                     start=(j % BPG == 0),
                                     stop=(j % BPG == BPG - 1))
                pooledT = tpool.tile([D, GPC], F32, tag="pooledT")
                nc.scalar.activation(pooledT[:], psp[:],
                                     mybir.ActivationFunctionType.Copy,
                                     scale=1.0 / K)

                psl = ps_mm.tile([GPC, OUT], F32, tag="mm")
                nc.tensor.matmul(psl[:], lhsT=pooledT[:], rhs=Wlsb[:],
                                 start=True, stop=True)
                lg = tpool.tile([GPC, OUT], F32, tag="lg")
                nc.vector.tensor_add(lg[:], psl[:], blsb[:GPC, :])
                mx = tpool.tile([GPC, 1], F32, tag="mx")
                nc.vector.tensor_reduce(mx[:], lg[:],
                                        axis=mybir.AxisListType.X,
                                        op=mybir.AluOpType.max)
                nmx = tpool.tile([GPC, 1], F32, tag="nmx")
                nc.vector.tensor_scalar_mul(nmx[:], mx[:], -1.0)
                ex = tpool.tile([GPC, OUT], F32, tag="ex")
                nc.scalar.activation(ex[:], lg[:],
                                     mybir.ActivationFunctionType.Exp,
                                     bias=nmx[:])
                se = tpool.tile([GPC, 1], F32, tag="se")
                nc.vector.tensor_reduce(se[:], ex[:],
                                        axis=mybir.AxisListType.X,
                                        op=mybir.AluOpType.add)
                ls = tpool.tile([GPC, 1], F32, tag="ls")
                nc.scalar.activation(ls[:], se[:],
                                     mybir.ActivationFunctionType.Ln)
                m2 = tpool.tile([GPC, 1], F32, tag="m2")
                nc.vector.tensor_add(m2[:], mx[:], ls[:])
                res = tpool.tile([GPC, OUT], F32, tag="res")
                nc.vector.tensor_sub(res[:], lg[:],
                                     m2[:].to_broadcast([GPC, OUT]))
                nc.sync.dma_start(out=outp[:], in_=res[:])

    nc.compile()
    return nc


class _Runner:
    """Caches the jitted PJRT executable for repeated invocations.

    Mirrors bass2jax.run_bass_via_pjrt's multi-core path, but keeps the
    jitted shard_map callable (and optionally device-resident inputs) so
    repeat calls skip retracing/relowering and input re-upload.
    """

    def __init__(self, nc):
        import jax
        from jax.sharding import Mesh, PartitionSpec, NamedSharding
        from jax.experimental.shard_map import shard_map
        from concourse import bass2jax

        bass2jax.install_neuronx_cc_hook()
        self.jax = jax
        self.nc = nc
        partition_name = (nc.partition_id_tensor.name
                          if nc.partition_id_tensor else None)
        in_names, out_names, out_avals, zero_outs = [], [], [], []
        for alloc in nc.m.functions[0].allocations:
            if not isinstance(alloc, mybir.MemoryLocationSet):
                continue
            name = alloc.memorylocations[0].name
            if alloc.kind == "ExternalInput":
                if name != partition_name:
                    in_names.append(name)
            elif alloc.kind == "ExternalOutput":
                shape = tuple(alloc.tensor_shape)
                dtype = mybir.dt.np(alloc.dtype)
                out_names.append(name)
                out_avals.append(jax.core.ShapedArray(shape, dtype))
                zero_outs.append(np.zeros(shape, dtype))
        self.in_names = list(in_names)
        self.out_names = out_names
        self.out_avals = out_avals
        self.zero_outs = zero_outs
        n_params = len(in_names)
        n_outs = len(out_names)
        all_in_names = in_names + out_names
        if partition_name is not None:
            all_in_names = all_in_names + [partition_name]

        def _body(*args):
            operands = list(args)
            if partition_name is not None:
                operands.append(bass2jax.partition_id_tensor())
            outs = bass2jax._bass_exec_p.bind(
                *operands,
                out_avals=tuple(out_avals),
                in_names=tuple(all_in_names),
                out_names=tuple(out_names),
                lowering_input_output_aliases=(),
                sim_require_finite=True,
                sim_require_nnan=True,
                nc=nc,
            )
            return tuple(outs)

        devices = jax.devices()[:NCORES]
        self.mesh = Mesh(np.asarray(devices), ("core",))
        self.sharding = NamedSharding(self.mesh, PartitionSpec("core"))
        in_specs = (PartitionSpec("core"),) * (n_params + n_outs)
        out_specs = (PartitionSpec("core"),) * n_outs
        self.fn = jax.jit(
            shard_map(_body, mesh=self.mesh, in_specs=in_specs,
                      out_specs=out_specs, check_rep=False),
            donate_argnums=tuple(range(n_params, n_params + n_outs)),
            keep_unused=True,
        )

    def concat_inputs(self, in_maps):
        return [
            np.concatenate([np.asarray(in_maps[c][name])
                            for c in range(NCORES)], axis=0)
            for name in self.in_names
        ]

    def device_put_inputs(self, concat_in):
        return [self.jax.device_put(a, self.sharding) for a in concat_in]

    def __call__(self, concat_in):
        zeros = [np.zeros((NCORES * z.shape[0], *z.shape[1:]), z.dtype)
                 for z in self.zero_outs]
        out_arrs = self.fn(*concat_in, *zeros)
        out_arrs = [np.asarray(a) for a in out_arrs]
        return [
            {name: out_arrs[i].reshape(NCORES, *self.out_avals[i].shape)[c]
             for i, name in enumerate(self.out_names)}
            for c in range(NCORES)
        ]


_CACHE = {}


def _get_runner(C_j, TC, zero_bias=False):
    key = (BLK, C_j, TC, str(DT_TBL), str(DT_OH), zero_bias)
    if key not in _CACHE:
        _CACHE[key] = _Runner(_build_program(C_j, TC, zero_bias=zero_bias))
    return _CACHE[key]


def make_in_maps(x, edge_index, W0, b0, W1, b1, pool_w, W_lin, b_lin):
    np_w = mybir.dt.np(DT_W)
    np_t = mybir.dt.np(DT_TBL)
    idx_row, col_lcl, C_j, TC, dis = _preprocess(np.asarray(edge_index))

    x = np.asarray(x, np.float32)
    x_pad = np.zeros((NPAD, D), np.float32)
    x_pad[:N] = x
    xs = (x_pad * dis[:, None]).astype(np_t)       # pre-scaled gather table

    pw = np.asarray(pool_w, np.float32)
    pwn = (pw / np.linalg.norm(pw)).astype(np.float32)

    def bc(v, n):
        return np.ascontiguousarray(
            np.broadcast_to(np.asarray(v, np.float32), (128, n)))

    common = {
        "W0": np.asarray(W0, np.float32).astype(np_w),
        "W1": np.asarray(W1, np.float32).astype(np_w),
        "Wl": np.asarray(W_lin, np.float32),
        "b0b": bc(b0, D),
        "b1b": bc(b1, D),
        "pwb": bc(pwn, D),
        "blb": bc(b_lin, OUT),
    }
    in_maps = []
    for k in range(NCORES):
        dis_k = dis[k * NLOC:(k + 1) * NLOC].reshape(LBLK, BLK).T
        in_maps.append(dict(
            common, xs=xs,
            dislT=np.ascontiguousarray(dis_k),
            idxs=np.ascontiguousarray(idx_row[k]),
            cols=np.ascontiguousarray(col_lcl[k]).astype(np_t)))
    zero_bias = (not np.any(np.asarray(b0))) and (not np.any(np.asarray(b1)))
    return in_maps, C_j, TC, zero_bias


def kernel(x, edge_index, batch, W0, b0, W1, b1, pool_w, W_lin, b_lin):
    in_maps, C_j, TC, zb = make_in_maps(x, edge_index, W0, b0, W1, b1,
                                        pool_w, W_lin, b_lin)
    runner = _get_runner(C_j, TC, zb)
    res = runner(runner.concat_inputs(in_maps))
    out = np.concatenate([res[k]["out"] for k in range(NCORES)], axis=0)
    return np.ascontiguousarray(out[:G])


# revision 58
# speedup vs baseline: 1.1145x; 1.1145x over previous
"""Trainium2 Bass kernel for a 2-layer GCN + TopK pooling + mean pool + linear head.

Reference computation (see problem):
  x = relu(gcn_conv(x, edge_index, W0, b0))
  x = relu(gcn_conv(x, edge_index, W1, b1))
  score = tanh((x @ pool_w) / ||pool_w||); top-K=250 of 500 per graph
  pooled = mean over kept nodes of (x * score); logits = pooled @ W_lin + b_lin
  out = log_softmax(logits)

Sharding: data-parallel over graphs. 104 padded graphs, 13 per core.
Each core aggregates (gather + one-hot matmul scatter) only the edges whose
*target* node lives in its 6500-node slab. Self-loops are appended to the
edge list so the GCN "+I" term needs no special case.

v2 design notes (vs the first working version):
  - Degrees and D^-1/2 factors are host-precomputed. Layer 1 gathers rows of
    xs = dis*x directly (W0 is applied AFTER aggregation: diag(dis)(A+I)
    diag(dis) (x W0) = [diag(dis)(A+I)(diag(dis) x)] W0), killing the
    on-device degree pass, the degree AllGather and the g1 table round-trip.
  - One-hot scatter matrices are built one DVE instruction per *block*
    (C chunks wide) instead of per chunk, via broadcast access patterns.
  - Edges are sorted by source within each (block, chunk) bucket so the
    indirect-gather descriptors hit adjacent DRAM rows when possible.
Cross-core exchange: one AllGather of the layer-2 gather table ([52000,64]).
"""

import os
import sys

for _p in ("/opt/trn_rl_repo", "/root/.axon_site/_ro/trn_rl_repo"):
    if os.path.isdir(_p) and _p not in sys.path:
        sys.path.insert(0, _p)

import dataclasses

import numpy as np

import concourse.bacc as bacc
import concourse.bass as bass
import concourse.mybir as mybir
import concourse.tile as tile
from concourse.bass_utils import run_bass_kernel_spmd  # noqa: F401  (public API)
from concourse.masks import make_identity
from concourse.tile import add_dep_helper


def _dep(after, befores):
    for b in befores:
        add_dep_helper(after.ins, b.ins, sync=True, reason="dram raw order")

# ---- problem constants (hardcoded per contract) ----
N = 50000          # real nodes
E = 800000         # edges
G = 100            # graphs
NPG = 500          # nodes per graph
K = 250            # top-k per graph
D = 64
OUT = 10
NCORES = 8
BLK = int(os.environ.get("GNN_BLK", "50"))  # nodes per aggregation block
GPC = 13           # graphs per core (padded to 104 graphs)
NPAD = NCORES * GPC * NPG      # 52000
NLOC = NPAD // NCORES          # 6500
LBLK = NLOC // BLK             # local blocks per core (130 @ BLK=50)
NBLK = NCORES * LBLK           # global blocks
CH = 128                       # edges per chunk (matmul contraction size)
QB = NPG // BLK                # blocks per group = one graph per group
BPG = NPG // BLK               # blocks per graph
WSUB = QB if QB * D * 4 <= 2048 else QB // 2   # W-matmul psum sub-group

F32 = mybir.dt.float32
I32 = mybir.dt.int32

# gather-table / one-hot dtype: float32 (exact) or bfloat16 (2x DMA, 4x PE)
DT_TBL = mybir.dt.bfloat16 if os.environ.get("GNN_TBL_BF16", "1") == "1" else F32
# weight dtype for the dense matmuls
DT_W = mybir.dt.bfloat16 if os.environ.get("GNN_W_BF16", "1") == "1" else F32
# one-hot matrix dtype (0/1 are exact in every float dtype; fp8 halves the
# SBUF footprint so the full set fits and layer 2 can reuse layer 1's)
DT_OH = (mybir.dt.float8e4 if os.environ.get("GNN_OH_FP8", "1") == "1"
         else DT_TBL)
# layer-2 exchange table dtype: fp8 halves the AllGather volume; the ~0.8%
# quantization noise averages out over ~16-neighbor aggregation + mean pool
DT_G2 = (mybir.dt.float8e4 if os.environ.get("GNN_G2_FP8", "1") == "1"
         else DT_TBL)


def _preprocess(edge_index):
    """Bucket edges (plus self-loops) by target block; build per-core
    [128, TC] index/column arrays laid out chunk-major. Also compute the
    symmetric-normalization factors dis = (deg)^-1/2 on the host."""
    row = np.asarray(edge_index[0], dtype=np.int64)
    col = np.asarray(edge_index[1], dtype=np.int64)
    loops = np.arange(NPAD, dtype=np.int64)
    rows_all = np.concatenate([row, loops])
    cols_all = np.concatenate([col, loops])

    deg = np.bincount(cols_all, minlength=NPAD).astype(np.float64)
    dis = (1.0 / np.sqrt(deg)).astype(np.float32)   # deg >= 1 (self-loops)

    blk = (cols_all // BLK).astype(np.int64)          # global target block
    col_loc = (cols_all % BLK).astype(np.int64)

    # sort by (block, source) - source-sorted order improves DMA locality
    order = np.lexsort((rows_all, blk))
    row_s = rows_all[order]
    colloc_s = col_loc[order]

    counts = np.bincount(blk, minlength=NBLK)
    cnts = counts.reshape(NCORES, LBLK)
    C_j = np.maximum(1, -(-cnts.max(axis=0) // CH))   # chunks per local block
    TC = int(C_j.sum())
    starts = np.zeros(LBLK, np.int64)
    starts[1:] = np.cumsum(C_j)[:-1]

    idx_row = np.zeros((NCORES, 128, 2 * TC), np.int32)
    col_lcl = np.full((NCORES, 128, TC), float(BLK), np.float32)  # pad -> no match
    bounds = np.concatenate([[0], np.cumsum(counts)])
    blk_sorted = blk[order]
    rank = np.arange(len(blk_sorted)) - bounds[blk_sorted]  # rank within block
    kk = blk_sorted // LBLK
    jj = blk_sorted % LBLK
    pp = rank % CH
    cc = starts[jj] + rank // CH
    idx_row[kk, pp, cc] = row_s            # layer-1 gather: rows of xs
    # layer-2 gather: rows of g2t, whose halves are AllGathered separately
    # (g2t[:NPAD//2] = concat_c g2l_c[:H], g2t[NPAD//2:] = concat_c g2l_c[H:])
    H = NLOC // 2
    c_src = row_s // NLOC
    n_loc = row_s % NLOC
    row2 = np.where(n_loc < H, c_src * H + n_loc,
                    NPAD // 2 + c_src * H + (n_loc - H))
    idx_row[kk, pp, TC + cc] = row2
    col_lcl[kk, pp, cc] = colloc_s
    return idx_row, col_lcl, tuple(int(c) for c in C_j), TC, dis


def _topk_mask(tc, out, in_, k_to_choose, min_val):
    """Mask of 1s where the top-k values per partition are (from
    concourse.kernels.top_k, inlined to fix a decorator/signature clash)."""
    nc = tc.nc
    KA = 8
    with tc.tile_pool(name="topk_sbuf", bufs=2) as sbuf_pool:
        tensor_on = in_
        for k_on in range(0, k_to_choose, KA):
            k_max = min(k_on + KA, k_to_choose)
            k_this = k_max - k_on
            mx = sbuf_pool.tile([in_.shape[0], KA], in_.dtype, tag="topk_mx")
            nc.vector.max(out=mx[:], in_=tensor_on)
            if k_this < KA:
                nc.vector.memset(mx[:, k_this:], min_val)
            nc.vector.match_replace(out=out, in_to_replace=mx[:],
                                    in_values=tensor_on, imm_value=min_val)
            tensor_on = out
        nc.vector.tensor_sub(out=out, in0=in_, in1=out)
        nc.vector.tensor_scalar_min(out, out, 1.0)


def _build_program(C_j, TC, sim=False, reps=1, zero_bias=False):
    # sim=True: single-core timing-model build - collectives replaced by
    # local DMA copies (TimelineSim can't model collectives).
    nc = bacc.Bacc("TRN2", target_bir_lowering=False, debug=False,
                   num_devices=1 if sim else NCORES)

    xs = nc.dram_tensor("xs", [NPAD, D], DT_TBL, kind="ExternalInput").ap()
    W0 = nc.dram_tensor("W0", [D, D], DT_W, kind="ExternalInput").ap()
    W1 = nc.dram_tensor("W1", [D, D], DT_W, kind="ExternalInput").ap()
    Wl = nc.dram_tensor("Wl", [D, OUT], F32, kind="ExternalInput").ap()
    b0b = nc.dram_tensor("b0b", [128, D], F32, kind="ExternalInput").ap()
    b1b = nc.dram_tensor("b1b", [128, D], F32, kind="ExternalInput").ap()
    pwb = nc.dram_tensor("pwb", [128, D], F32, kind="ExternalInput").ap()
    blb = nc.dram_tensor("blb", [128, OUT], F32, kind="ExternalInput").ap()
    dislT = nc.dram_tensor("dislT", [BLK, LBLK], F32, kind="ExternalInput").ap()
    idxs = nc.dram_tensor("idxs", [128, 2 * TC], I32, kind="ExternalInput").ap()
    cols = nc.dram_tensor("cols", [128, TC], DT_TBL, kind="ExternalInput").ap()
    outp = nc.dram_tensor("out", [GPC, OUT], F32, kind="ExternalOutput").ap()

    g2l = nc.dram_tensor("g2l", [NLOC, D], DT_G2,
                         kind="ExternalOutput" if os.environ.get("GNN_DBG_G2")
                         else "Internal").ap()
    g2t = nc.dram_tensor("g2t", [NPAD, D], DT_G2, kind="Internal",
                         addr_space="Shared").ap()
    scd = nc.dram_tensor("scd", [NLOC], DT_TBL, kind="Internal").ap()
    wd = nc.dram_tensor("wd", [NLOC], DT_TBL, kind="Internal").ap()

    starts = [0] * LBLK
    for j in range(1, LBLK):
        starts[j] = starts[j - 1] + C_j[j - 1]
    Cmax = max(C_j)

    rg = [list(range(NCORES))]

    def bcast_mid(ap2d, nmid):
        """[P, W] tile -> [P, nmid, W] AP with step-0 middle dim."""
        a = ap2d.ap
        return dataclasses.replace(ap2d, ap=[list(a[0]), [0, nmid],
                                             list(a[1])])

    with tile.TileContext(nc) as tc:
        with (
            tc.tile_pool(name="const", bufs=1) as cpool,
            tc.tile_pool(name="slab", bufs=1) as slab,
            tc.tile_pool(name="gat", bufs=6) as gatpool,
            tc.tile_pool(name="tmp", bufs=4) as tpool,
            tc.tile_pool(name="ps_agg", bufs=3, space="PSUM") as ps_agg,
            tc.tile_pool(name="ps_mm", bufs=2, space="PSUM") as ps_mm,
            tc.tile_pool(name="ps_tr", bufs=2, space="PSUM") as ps_tr,
            tc.tile_pool(name="ps_acc", bufs=1, space="PSUM") as ps_acc,
        ):
            # ---- constants (tiles here; DMAs issued after the gather
            #      index tables so the first gather isn't queued behind them)
            W0sb = cpool.tile([D, D], DT_W)
            W1sb = cpool.tile([D, D], DT_W)
            Wlsb = cpool.tile([D, OUT], F32)
            b0sb = cpool.tile([128, D], F32)
            b1sb = cpool.tile([128, D], F32)
            pwsb = cpool.tile([128, D], F32)
            blsb = cpool.tile([128, OUT], F32)
            dissb = cpool.tile([BLK, LBLK], F32)

            for _rep in range(reps):
                idx1_sb = slab.tile([128, TC], I32)
                idx2_sb = slab.tile([128, TC], I32)
                col_sb = slab.tile([128, TC], DT_TBL)
                nc.sync.dma_start(out=idx1_sb[:], in_=idxs[:, :TC])
                nc.sync.dma_start(out=col_sb[:], in_=cols[:])
                nc.sync.dma_start(out=idx2_sb[:], in_=idxs[:, TC:])
                if _rep == 0:
                    nc.sync.dma_start(out=W0sb[:], in_=W0[:])
                    nc.sync.dma_start(out=dissb[:], in_=dislT[:])
                    nc.sync.dma_start(out=W1sb[:], in_=W1[:])
                    nc.sync.dma_start(out=b0sb[:], in_=b0b[:])
                    nc.sync.dma_start(out=b1sb[:], in_=b1b[:])
                    nc.sync.dma_start(out=pwsb[:], in_=pwb[:])
                    nc.sync.dma_start(out=Wlsb[:], in_=Wl[:])
                    nc.sync.dma_start(out=blsb[:], in_=blb[:])

                iota_i = cpool.tile([128, BLK], I32)
                iota_f = cpool.tile([128, BLK], DT_TBL)
                nc.gpsimd.iota(iota_i[:], pattern=[[1, BLK]], base=0,
                               channel_multiplier=0)
                nc.vector.tensor_copy(iota_f[:], iota_i[:])
                ones_f = cpool.tile([128, 1], F32)
                nc.vector.memset(ones_f[:], 1.0)
                ident = cpool.tile([128, 128], F32)
                make_identity(nc, ident[:])

                aggT = slab.tile([D, NLOC], DT_TBL)       # transposed agg
                g2slab = slab.tile([BLK, LBLK * D], DT_G2)
                out2_slab = slab.tile([BLK, LBLK * D], F32)
                sc_slab = slab.tile([BLK, LBLK], F32)
                dissq = cpool.tile([BLK, LBLK], F32)      # dis^2 per node
                nc.vector.tensor_mul(dissq[:], dissb[:], dissb[:])

                oh_full = slab.tile([128, TC * BLK], DT_OH)

                def layer_pass(table, idx_sb, table_deps, Wsb, post_cb,
                               build_oh, gat_dt=DT_TBL):
                    """Per group of QB blocks: gather rows of `table`,
                    scatter-sum them via one-hot matmuls (gathered chunk is
                    the stationary operand -> cheap LDWEIGHTS; the [64, BLK]
                    product is the aggregation pre-transposed, exactly the
                    lhsT the W matmul wants), then apply W and hand the psum
                    to post_cb(jq, psh). disl scaling happens in post_cb."""
                    for jq in range(0, LBLK, QB):
                        o0 = starts[jq]
                        ctot = sum(C_j[jq:jq + QB])
                        gat = gatpool.tile([128, QB * Cmax * D], gat_dt,
                                           tag="gat")
                        g_ins = nc.gpsimd.indirect_dma_start(
                            out=gat[:, :ctot * D],
                            out_offset=None,
                            in_=table[:],
                            in_offset=bass.IndirectOffsetOnAxis(
                                ap=idx_sb[:, o0:o0 + ctot], axis=0),
                        )
                        _dep(g_ins, table_deps)
                        # one-hot scatter matrices (shared by both layers:
                        # same edges), one DVE instruction per block
                        if build_oh:
                            for bi in range(QB):
                                j = jq + bi
                                cj = C_j[j]
                                nc.vector.tensor_tensor(
                                    out=oh_full[:, starts[j] * BLK:
                                                (starts[j] + cj) * BLK]
                                        .rearrange("p (c b) -> p c b", b=BLK),
                                    in0=col_sb[:, starts[j]:starts[j] + cj]
                                        .to_broadcast([128, cj, BLK]),
                                    in1=bcast_mid(iota_f[:], cj),
                                    op=mybir.AluOpType.is_equal)
                        ps4 = ps_agg.tile([D, QB * BLK], F32, tag="agg")
                        for bi in range(QB):
                            j = jq + bi
                            coff = starts[j] - o0
                            for c in range(C_j[j]):
                                gc = starts[j] + c
                                nc.tensor.matmul(
                                    ps4[:, bi * BLK:(bi + 1) * BLK],
                                    lhsT=gat[:, (coff + c) * D:
                                             (coff + c + 1) * D],
                                    rhs=oh_full[:, gc * BLK:
                                                (gc + 1) * BLK],
                                    start=(c == 0), stop=(c == C_j[j] - 1))
                        # psum -> sbuf copy on the (mostly idle) scalar
                        # engine so the DVE FIFO never head-blocks on it
                        nc.scalar.activation(
                            aggT[:, jq * BLK:(jq + QB) * BLK], ps4[:],
                            mybir.ActivationFunctionType.Copy)
                        pshs = []
                        for s0 in range(0, QB, WSUB):
                            psh = ps_mm.tile([BLK, WSUB * D], F32, tag="mm")
                            for bi in range(WSUB):
                                j = jq + s0 + bi
                                nc.tensor.matmul(
                                    psh[:, bi * D:(bi + 1) * D],
                                    lhsT=aggT[:, j * BLK:(j + 1) * BLK],
                                    rhs=Wsb[:], start=True, stop=True)
                            pshs.append((jq + s0, psh))
                        post_cb(jq, pshs)

                def scale_dis(dst_ap, src_ap, sq):
                    nc.vector.tensor_tensor(
                        out=dst_ap.rearrange("p (b d) -> p b d", d=D),
                        in0=src_ap.rearrange("p (b d) -> p b d", d=D),
                        in1=dissb[:, sq:sq + WSUB].to_broadcast(
                            [BLK, WSUB, D]),
                        op=mybir.AluOpType.mult)

                def add_bias(dst_ap, src_ap, bsb):
                    nc.vector.tensor_tensor(
                        out=dst_ap.rearrange("p (b d) -> p b d", d=D),
                        in0=src_ap.rearrange("p (b d) -> p b d", d=D),
                        in1=bcast_mid(bsb[:BLK, :], WSUB),
                        op=mybir.AluOpType.add)

                # ---- layer 1: agg = (A+I) xs (xs = dis*x pre-scaled);
                #      out1 = relu(dis*agg W0 + b0); g2 = dis * out1.
                #      The g2 AllGather is split in halves so the first
                #      half's exchange overlaps the second half's compute.
                HALF = LBLK // 2
                g2_stores = []
                ag_list = []

                def l1_post(jq, pshs):
                    for sq, psh in pshs:
                        hb_sb = tpool.tile([BLK, WSUB * D], F32, tag="hb")
                        if zero_bias:
                            # g2 = dis * relu(dis * (agg W0)) =
                            #      dis^2 * relu(agg W0)   (dis > 0)
                            nc.scalar.activation(
                                hb_sb[:], psh[:],
                                mybir.ActivationFunctionType.Relu)
                            nc.vector.tensor_tensor(
                                out=g2slab[:, sq * D:(sq + WSUB) * D]
                                    .rearrange("p (b d) -> p b d", d=D),
                                in0=hb_sb[:].rearrange(
                                    "p (b d) -> p b d", d=D),
                                in1=dissq[:, sq:sq + WSUB].to_broadcast(
                                    [BLK, WSUB, D]),
                                op=mybir.AluOpType.mult)
                            continue
                        scale_dis(hb_sb[:], psh[:], sq)
                        add_bias(hb_sb[:], hb_sb[:], b0sb)
                        nc.scalar.activation(
                            hb_sb[:], hb_sb[:],
                            mybir.ActivationFunctionType.Relu)
                        nc.vector.tensor_tensor(
                            out=g2slab[:, sq * D:(sq + WSUB) * D].rearrange(
                                "p (b d) -> p b d", d=D),
                            in0=hb_sb[:].rearrange("p (b d) -> p b d", d=D),
                            in1=dissb[:, sq:sq + WSUB].to_broadcast(
                                [BLK, WSUB, D]),
                            op=mybir.AluOpType.mult)
                    g2_stores.append(nc.sync.dma_start(
                        out=g2l.rearrange("(b p) d -> p b d", p=BLK)
                            [:, jq:jq + QB, :],
                        in_=g2slab[:, jq * D:(jq + QB) * D].rearrange(
                            "p (b d) -> p b d", d=D)))
                    sq = jq
                    if sq < HALF <= sq + QB:  # first-half blocks all stored
                        if sim:
                            ag = nc.gpsimd.dma_start(
                                out=g2t[:HALF * BLK, :],
                                in_=g2l[:HALF * BLK, :])
                        else:
                            ag = nc.gpsimd.collective_compute(
                                "AllGather", mybir.AluOpType.bypass,
                                replica_groups=rg,
                                ins=[g2l[:HALF * BLK, :]],
                                outs=[g2t[:NPAD // 2, :]])
                        _dep(ag, g2_stores)
                        ag_list.append(ag)

                layer_pass(xs, idx1_sb, [], W0sb, l1_post, build_oh=True)
                if sim:
                    ag = nc.gpsimd.dma_start(out=g2t[HALF * BLK:NLOC, :],
                                             in_=g2l[HALF * BLK:, :])
                else:
                    ag = nc.gpsimd.collective_compute(
                        "AllGather", mybir.AluOpType.bypass,
                        replica_groups=rg, ins=[g2l[HALF * BLK:, :]],
                        outs=[g2t[NPAD // 2:, :]])
                _dep(ag, g2_stores)
                ag_list.append(ag)

                # ---- layer 2: agg2 = (A+I) g2 ; out2 = relu(dis*agg2 W1+b1)
                #      (scores are computed per group so they overlap with
                #      the remaining groups' aggregation work)
                sc_view = scd.rearrange("(j p) -> j p", p=BLK)
                w_stores = []

                def l2_post(jq, pshs):
                    for sq, psh in pshs:
                        if zero_bias:
                            hb_sb = tpool.tile([BLK, WSUB * D], F32,
                                               tag="hb")
                            nc.scalar.activation(
                                hb_sb[:], psh[:],
                                mybir.ActivationFunctionType.Relu)
                            scale_dis(
                                out2_slab[:, sq * D:(sq + WSUB) * D],
                                hb_sb[:], sq)
                        else:
                            scale_dis(out2_slab[:, sq * D:(sq + WSUB) * D],
                                      psh[:], sq)
                            add_bias(out2_slab[:, sq * D:(sq + WSUB) * D],
                                     out2_slab[:, sq * D:(sq + WSUB) * D],
                                     b1sb)
                            nc.scalar.activation(
                                out2_slab[:, sq * D:(sq + WSUB) * D],
                                out2_slab[:, sq * D:(sq + WSUB) * D],
                                mybir.ActivationFunctionType.Relu)
                    # per-group scores: sc[p, j] = sum_d out2[p, j, d]*pw[d]
                    grp = out2_slab[:, jq * D:(jq + QB) * D]
                    sct = tpool.tile([BLK, QB * D], F32, tag="sct")
                    nc.vector.tensor_tensor(
                        out=sct[:].rearrange("p (b d) -> p b d", d=D),
                        in0=grp.rearrange("p (b d) -> p b d", d=D),
                        in1=bcast_mid(pwsb[:BLK, :], QB),
                        op=mybir.AluOpType.mult)
                    nc.vector.tensor_reduce(
                        sc_slab[:, jq:jq + QB],
                        sct[:].rearrange("p (b d) -> p b d", d=D),
                        axis=mybir.AxisListType.X, op=mybir.AluOpType.add)
                layer_pass(g2t, idx2_sb, ag_list, W1sb, l2_post,
                           build_oh=False, gat_dt=DT_G2)
                # single end-of-layer topk: injecting it mid-L2 delays the
                # DVE one-hot builds that pace the pipeline (measured slower)
                HJ = LBLK // 2
                nc.scalar.activation(sc_slab[:], sc_slab[:],
                                     mybir.ActivationFunctionType.Tanh)
                pst = ps_tr.tile([HJ, 2 * BLK], F32, tag="tr")
                nc.tensor.transpose(pst[:, :BLK], sc_slab[:, :HJ],
                                    ident[:BLK, :BLK])
                nc.tensor.transpose(pst[:, BLK:], sc_slab[:, HJ:],
                                    ident[:BLK, :BLK])
                scT = tpool.tile([HJ, 2 * BLK], DT_TBL, tag="scT")
                nc.vector.tensor_copy(scT[:], pst[:])
                st_sc = [
                    nc.sync.dma_start(out=sc_view[:HJ], in_=scT[:, :BLK]),
                    nc.sync.dma_start(out=sc_view[HJ:], in_=scT[:, BLK:]),
                ]
                sc13 = slab.tile([GPC, NPG], DT_TBL)
                ld_sc = nc.sync.dma_start(
                    out=sc13[:], in_=scd.rearrange("(g n) -> g n", n=NPG))
                _dep(ld_sc, st_sc)
                mask13 = slab.tile([GPC, NPG], DT_TBL)
                _topk_mask(tc, mask13[:], sc13[:], K, min_val=-4.0)
                w13 = slab.tile([GPC, NPG], DT_TBL)
                nc.vector.tensor_mul(w13[:], mask13[:], sc13[:])
                w_stores.append(nc.sync.dma_start(
                    out=wd.rearrange("(g n) -> g n", n=NPG), in_=w13[:]))
                wT = tpool.tile([HJ, 2 * BLK], DT_TBL, tag="wT")
                w_view = wd.rearrange("(j p) -> j p", p=BLK)
                ld_w = [
                    nc.sync.dma_start(out=wT[:, :BLK], in_=w_view[:HJ]),
                    nc.sync.dma_start(out=wT[:, BLK:], in_=w_view[HJ:]),
                ]
                _dep(ld_w[0], w_stores)
                _dep(ld_w[1], w_stores)
                wTf = tpool.tile([HJ, 2 * BLK], F32, tag="wTf")
                nc.vector.tensor_copy(wTf[:], wT[:])
                psw = ps_tr.tile([BLK, LBLK], F32, tag="tr")
                nc.tensor.transpose(psw[:, :HJ], wTf[:, :BLK],
                                    ident[:HJ, :HJ])
                nc.tensor.transpose(psw[:, HJ:], wTf[:, BLK:],
                                    ident[:HJ, :HJ])
                w_slab = slab.tile([BLK, LBLK], F32)
                nc.vector.tensor_copy(w_slab[:], psw[:])

                # pooledT[d, gr] = sum_n w[n] * out2[n, d]: the w multiply
                # rides in the matmul as the [BLK, 1] moving operand
                psp = ps_acc.tile([D, GPC], F32, tag="acc")
                for j in range(LBLK):
                    gr = j // BPG
                    nc.tensor.matmul(psp[:, gr:gr + 1],
                                     lhsT=out2_slab[:, j * D:(j + 1) * D],
                                     rhs=w_slab[:, j:j + 1],
                                     start=(j % BPG == 0),
                                     stop=(j % BPG == BPG - 1))
                pooledT = tpool.tile([D, GPC], F32, tag="pooledT")
                nc.scalar.activation(pooledT[:], psp[:],
                                     mybir.ActivationFunctionType.Copy,
                                     scale=1.0 / K)

                psl = ps_mm.tile([GPC, OUT], F32, tag="mm")
                nc.tensor.matmul(psl[:], lhsT=pooledT[:], rhs=Wlsb[:],
                                 start=True, stop=True)
                lg = tpool.tile([GPC, OUT], F32, tag="lg")
                nc.vector.tensor_add(lg[:], psl[:], blsb[:GPC, :])
                mx = tpool.tile([GPC, 1], F32, tag="mx")
                nc.vector.tensor_reduce(mx[:], lg[:],
                                        axis=mybir.AxisListType.X,
                                        op=mybir.AluOpType.max)
                nmx = tpool.tile([GPC, 1], F32, tag="nmx")
                nc.vector.tensor_scalar_mul(nmx[:], mx[:], -1.0)
                ex = tpool.tile([GPC, OUT], F32, tag="ex")
                nc.scalar.activation(ex[:], lg[:],
                                     mybir.ActivationFunctionType.Exp,
                                     bias=nmx[:])
                se = tpool.tile([GPC, 1], F32, tag="se")
                nc.vector.tensor_reduce(se[:], ex[:],
                                        axis=mybir.AxisListType.X,
                                        op=mybir.AluOpType.add)
                ls = tpool.tile([GPC, 1], F32, tag="ls")
                nc.scalar.activation(ls[:], se[:],
                                     mybir.ActivationFunctionType.Ln)
                m2 = tpool.tile([GPC, 1], F32, tag="m2")
                nc.vector.tensor_add(m2[:], mx[:], ls[:])
                res = tpool.tile([GPC, OUT], F32, tag="res")
                nc.vector.tensor_sub(res[:], lg[:],
                                     m2[:].to_broadcast([GPC, OUT]))
                nc.sync.dma_start(out=outp[:], in_=res[:])

    nc.compile()
    return nc


class _Runner:
    """Caches the jitted PJRT executable for repeated invocations.

    Mirrors bass2jax.run_bass_via_pjrt's multi-core path, but keeps the
    jitted shard_map callable (and optionally device-resident inputs) so
    repeat calls skip retracing/relowering and input re-upload.
    """

    def __init__(self, nc):
        import jax
        from jax.sharding import Mesh, PartitionSpec, NamedSharding
        from jax.experimental.shard_map import shard_map
        from concourse import bass2jax

        bass2jax.install_neuronx_cc_hook()
        self.jax = jax
        self.nc = nc
        partition_name = (nc.partition_id_tensor.name
                          if nc.partition_id_tensor else None)
        in_names, out_names, out_avals, zero_outs = [], [], [], []
        for alloc in nc.m.functions[0].allocations:
            if not isinstance(alloc, mybir.MemoryLocationSet):
                continue
            name = alloc.memorylocations[0].name
            if alloc.kind == "ExternalInput":
                if name != partition_name:
                    in_names.append(name)
            elif alloc.kind == "ExternalOutput":
                shape = tuple(alloc.tensor_shape)
                dtype = mybir.dt.np(alloc.dtype)
                out_names.append(name)
                out_avals.append(jax.core.ShapedArray(shape, dtype))
                zero_outs.append(np.zeros(shape, dtype))
        self.in_names = list(in_names)
        self.out_names = out_names
        self.out_avals = out_avals
        self.zero_outs = zero_outs
        n_params = len(in_names)
        n_outs = len(out_names)
        all_in_names = in_names + out_names
        if partition_name is not None:
            all_in_names = all_in_names + [partition_name]

        def _body(*args):
            operands = list(args)
            if partition_name is not None:
                operands.append(bass2jax.partition_id_tensor())
            outs = bass2jax._bass_exec_p.bind(
                *operands,
                out_avals=tuple(out_avals),
                in_names=tuple(all_in_names),
                out_names=tuple(out_names),
                lowering_input_output_aliases=(),
                sim_require_finite=True,
                sim_require_nnan=True,
                nc=nc,
            )
            return tuple(outs)

        devices = jax.devices()[:NCORES]
        self.mesh = Mesh(np.asarray(devices), ("core",))
        self.sharding = NamedSharding(self.mesh, PartitionSpec("core"))
        in_specs = (PartitionSpec("core"),) * (n_params + n_outs)
        out_specs = (PartitionSpec("core"),) * n_outs
        self.fn = jax.jit(
            shard_map(_body, mesh=self.mesh, in_specs=in_specs,
                      out_specs=out_specs, check_rep=False),
            donate_argnums=tuple(range(n_params, n_params + n_outs)),
            keep_unused=True,
        )

    def concat_inputs(self, in_maps):
        return [
            np.concatenate([np.asarray(in_maps[c][name])
                            for c in range(NCORES)], axis=0)
            for name in self.in_names
        ]

    def device_put_inputs(self, concat_in):
        return [self.jax.device_put(a, self.sharding) for a in concat_in]

    def __call__(self, concat_in):
        zeros = [np.zeros((NCORES * z.shape[0], *z.shape[1:]), z.dtype)
                 for z in self.zero_outs]
        out_arrs = self.fn(*concat_in, *zeros)
        out_arrs = [np.asarray(a) for a in out_arrs]
        return [
            {name: out_arrs[i].reshape(NCORES, *self.out_avals[i].shape)[c]
             for i, name in enumerate(self.out_names)}
            for c in range(NCORES)
        ]


_CACHE = {}


def _get_runner(C_j, TC, zero_bias=False):
    key = (BLK, C_j, TC, str(DT_TBL), str(DT_OH), zero_bias)
    if key not in _CACHE:
        _CACHE[key] = _Runner(_build_program(C_j, TC, zero_bias=zero_bias))
    return _CACHE[key]


def make_in_maps(x, edge_index, W0, b0, W1, b1, pool_w, W_lin, b_lin):
    np_w = mybir.dt.np(DT_W)
    np_t = mybir.dt.np(DT_TBL)
    idx_row, col_lcl, C_j, TC, dis = _preprocess(np.asarray(edge_index))

    x = np.asarray(x, np.float32)
    x_pad = np.zeros((NPAD, D), np.float32)
    x_pad[:N] = x
    xs = (x_pad * dis[:, None]).astype(np_t)       # pre-scaled gather table

    pw = np.asarray(pool_w, np.float32)
    pwn = (pw / np.linalg.norm(pw)).astype(np.float32)

    def bc(v, n):
        return np.ascontiguousarray(
            np.broadcast_to(np.asarray(v, np.float32), (128, n)))

    common = {
        "W0": np.asarray(W0, np.float32).astype(np_w),
        "W1": np.asarray(W1, np.float32).astype(np_w),
        "Wl": np.asarray(W_lin, np.float32),
        "b0b": bc(b0, D),
        "b1b": bc(b1, D),
        "pwb": bc(pwn, D),
        "blb": bc(b_lin, OUT),
    }
    in_maps = []
    for k in range(NCORES):
        dis_k = dis[k * NLOC:(k + 1) * NLOC].reshape(LBLK, BLK).T
        in_maps.append(dict(
            common, xs=xs,
            dislT=np.ascontiguousarray(dis_k),
            idxs=np.ascontiguousarray(idx_row[k]),
            cols=np.ascontiguousarray(col_lcl[k]).astype(np_t)))
    zero_bias = (not np.any(np.asarray(b0))) and (not np.any(np.asarray(b1)))
    return in_maps, C_j, TC, zero_bias


def kernel(x, edge_index, batch, W0, b0, W1, b1, pool_w, W_lin, b_lin):
    in_maps, C_j, TC, zb = make_in_maps(x, edge_index, W0, b0, W1, b1,
                                        pool_w, W_lin, b_lin)
    runner = _get_runner(C_j, TC, zb)
    res = runner(runner.concat_inputs(in_maps))
    out = np.concatenate([res[k]["out"] for k in range(NCORES)], axis=0)
    return np.ascontiguousarray(out[:G])
